# Initial kernel scaffold
#
"""Trainium2 Bass kernel for nn_CLCRec contrastive loss.

Strategy (fully local per core, no collectives):
  - Batch rows are sharded 8 ways (2048 rows/core, 17 group entries each).
  - Host packs a combined per-item table  EV[j] = [id_emb[NUM_USER+j] | v_feat[j]]
    (192 f32 = 768B rows) so one indirect-DMA row fetch supplies both the raw
    item embedding (E) and the encoder input (v) for that batch entry.
  - Each core gathers EV rows for its batch entries, runs the MLP encoder on
    the gathered v-halves (PE matmuls), and keeps encoded features + E rows
    in SBUF (bf16).
  - User / positive-item embedding rows are gathered with indirect DMA (bf16).
  - Row-wise dot products + norms feed the two contrastive losses; per-core
    partial sums of the per-row log terms are returned and reduced on host.

Hardware notes baked in here:
  - Indirect DMA honours exactly ONE index per destination partition; multi-
    index offset tiles silently fetch consecutive rows.  All gathers use
    offset tiles of shape [128, 1].
  - All ACT functions used (Ln/Exp/Lrelu/Copy/Identity) live in the
    natural_log_exp_and_others table set; get_activation_tables is patched so
    the table-load pass cannot pick the per-anchor sets (which caused 261
    ACT_TABLE_LOADs, ~335us, in the unpatched build).
"""

import os
import sys

import numpy as np

for _p in ("/opt/trn_rl_repo", os.path.expanduser("~/.axon_site/_ro/trn_rl_repo")):
    if os.path.isdir(_p) and _p not in sys.path:
        sys.path.insert(0, _p)

import concourse.bacc as bacc
import concourse.mybir as mybir
import concourse.tile as tile
from concourse import bass_utils
from concourse.bass import IndirectOffsetOnAxis
from concourse.masks import make_identity

F32 = mybir.dt.float32
BF16 = mybir.dt.bfloat16
I32 = mybir.dt.int32
AF = mybir.ActivationFunctionType
ALU = mybir.AluOpType
AX = mybir.AxisListType

NUM_USER = 200000
NUM_ITEM = 200000
DIM_E = 64
DIM_FEAT = 128
EVW = DIM_E + DIM_FEAT   # 192 floats per packed EV row
B = 16384
G = 17  # 1 + num_neg
TEMP = 0.2
LR_LAMBDA = 0.5

NCORE = 8
BC = B // NCORE          # 2048 batch rows per core
NT = BC // 128           # 16 batch tiles (128 batch rows each)
CB = NT * G              # 272 column blocks of 128 rows per core
NCH = CB // 4            # 68 encoder chunks (512 gathered rows each)

_CACHE: dict = {}

_NEEDED_AF = None  # set lazily (mybir enums)


def _patch_act_tables():
    """Force every activation we emit to resolve to the combined
    natural_log_exp_and_others set so no per-anchor table swapping occurs."""
    global _NEEDED_AF
    if _CACHE.get("act_patched"):
        return
    _NEEDED_AF = {AF.Ln, AF.Exp, AF.Prelu, AF.Copy, AF.Identity}
    import concourse.hw_specs as hw_specs
    orig = hw_specs.get_activation_tables

    def patched(module_arch):
        tabs = orig(module_arch)
        out = {}
        for name, fns in tabs.items():
            if name == "natural_log_exp_and_others":
                out[name] = fns
            else:
                out[name] = fns - _NEEDED_AF
        return out

    bacc.get_activation_tables = patched
    _CACHE["act_patched"] = True


def _build():
    _patch_act_tables()
    nc = bacc.Bacc("TRN2", target_bir_lowering=False, debug=False, num_devices=NCORE)

    ev_d = nc.dram_tensor("evtab", [NUM_ITEM, EVW], F32, kind="ExternalInput")
    id_d = nc.dram_tensor("iduser", [NUM_USER, DIM_E], F32, kind="ExternalInput")
    w1_d = nc.dram_tensor("w1", [DIM_FEAT, 256], F32, kind="ExternalInput")
    b1_d = nc.dram_tensor("b1", [256], F32, kind="ExternalInput")
    w2_d = nc.dram_tensor("w2", [256, DIM_E], F32, kind="ExternalInput")
    b2_d = nc.dram_tensor("b2", [DIM_E], F32, kind="ExternalInput")
    iv_d = nc.dram_tensor("idxv", [NCH, 128, 4], I32, kind="ExternalInput")
    iu_d = nc.dram_tensor("idxu", [NT, 128, G], I32, kind="ExternalInput")
    ip_d = nc.dram_tensor("idxp", [NT, 128, 1], I32, kind="ExternalInput")
    mk_d = nc.dram_tensor("mask", [NT, 128, G], I32, kind="ExternalInput")
    out_d = nc.dram_tensor("acc_out", [128, 2], F32, kind="ExternalOutput")
    DEBUG = bool(int(os.environ.get("KERNEL_DEBUG", "0")))
    if DEBUG:
        dbgF_d = nc.dram_tensor("dbg_f", [128, CB * 64], F32, kind="ExternalOutput")
        dbg1_d = nc.dram_tensor("dbg_d1", [NT, 128, G], F32, kind="ExternalOutput")
        dbg2_d = nc.dram_tensor("dbg_d2", [NT, 128, G], F32, kind="ExternalOutput")

    with tile.TileContext(nc) as tc:
        with tc.tile_pool(name="const", bufs=1) as cp, \
             tc.tile_pool(name="fall", bufs=1) as fp, \
             tc.tile_pool(name="enc", bufs=3) as ep, \
             tc.tile_pool(name="ph2", bufs=3) as pp, \
             tc.tile_pool(name="prd", bufs=3) as prp, \
             tc.tile_pool(name="psT", bufs=3, space="PSUM") as psT, \
             tc.tile_pool(name="psH", bufs=3, space="PSUM") as psH, \
             tc.tile_pool(name="psF", bufs=2, space="PSUM") as psF:

            ident = cp.tile([128, 128], F32, tag="ident")
            make_identity(nc, ident[:])

            w1sb = cp.tile([128, 256], F32, tag="w1sb")
            nc.sync.dma_start(out=w1sb[:], in_=w1_d[:])
            w2sb = cp.tile([128, 128], F32, tag="w2sb")
            nc.sync.dma_start(out=w2sb[:, 0:64], in_=w2_d[0:128, :])
            nc.sync.dma_start(out=w2sb[:, 64:128], in_=w2_d[128:256, :])
            b1sb = cp.tile([128, 2], F32, tag="b1sb")
            nc.sync.dma_start(out=b1sb[:], in_=b1_d[:].rearrange("(h p) -> p h", p=128))
            b2sb = cp.tile([64, 1], F32, tag="b2sb")
            nc.sync.dma_start(out=b2sb[:], in_=b2_d[:].rearrange("(e o) -> e o", o=1))

            # Per-(batch row, group) encoded features and raw item embeddings,
            # kept in SBUF for the whole kernel (bf16 to fit).
            F_all = fp.tile([128, CB * 64], BF16, tag="fall")
            E_all = fp.tile([128, CB * 64], BF16, tag="eall")
            acc = cp.tile([128, 2], F32, tag="acc")
            nc.vector.memset(acc[:], 0.0)

            def enc_chunk(c):
                iv = ep.tile([128, 4], I32, tag="iv", bufs=6)
                nc.sync.dma_start(out=iv[:], in_=iv_d[c])
                ev = ep.tile([128, 4 * EVW], BF16, tag="ev", bufs=5)
                for j in range(4):
                    nc.gpsimd.indirect_dma_start(
                        out=ev[:, j * EVW:(j + 1) * EVW], out_offset=None,
                        in_=ev_d[:],
                        in_offset=IndirectOffsetOnAxis(ap=iv[:, j:j + 1], axis=0))
                ev3 = ev[:].rearrange("p (j e) -> p j e", e=EVW)
                v3 = ev3[:, :, DIM_E:EVW]          # [128, 4, 128] strided
                # stash raw item embedding halves (bf16) for the l2 loss
                nc.scalar.activation(
                    out=E_all[:, c * 256:(c + 1) * 256].rearrange(
                        "p (j e) -> p j e", e=64),
                    in_=ev3[:, :, 0:DIM_E], func=AF.Copy)
                sq = ep.tile([128, 512], F32, tag="sq")
                ss = ep.tile([128, 4], F32, tag="ss")
                nc.vector.tensor_tensor(
                    out=sq[:].rearrange("p (j e) -> p j e", e=DIM_FEAT),
                    in0=v3, in1=v3, op=ALU.mult)
                nc.vector.tensor_reduce(
                    out=ss[:], in_=sq[:].rearrange("p (j e) -> p j e", e=DIM_FEAT),
                    op=ALU.add, axis=AX.X)
                # 1/max(sqrt(ss), 1e-12) == rsqrt(max(ss, 1e-24)) = exp(-0.5*ln(.))
                nc.vector.tensor_scalar_max(out=ss[:], in0=ss[:], scalar1=1e-24)
                nc.scalar.activation(out=ss[:], in_=ss[:], func=AF.Ln)
                nc.scalar.activation(out=ss[:], in_=ss[:], func=AF.Exp, scale=-0.5)
                vn = ep.tile([128, 512], F32, tag="vn")
                nc.vector.tensor_tensor(
                    out=vn[:].rearrange("p (j e) -> p j e", e=DIM_FEAT),
                    in0=v3,
                    in1=ss[:].rearrange("p (j o) -> p j o", o=1).to_broadcast([128, 4, DIM_FEAT]),
                    op=ALU.mult)
                vT_ps = psT.tile([128, 512], F32, tag="tp")
                for j in range(4):
                    nc.tensor.transpose(
                        out=vT_ps[:, j * 128:(j + 1) * 128],
                        in_=vn[:, j * 128:(j + 1) * 128],
                        identity=ident[:])
                vT = ep.tile([128, 512], F32, tag="vT")
                nc.scalar.copy(out=vT[:], in_=vT_ps[:])
                h_sb = ep.tile([128, 1024], F32, tag="hsb")
                for h in range(2):
                    h_ps = psH.tile([128, 512], F32, tag="hp")
                    nc.tensor.matmul(
                        out=h_ps[:], lhsT=w1sb[:, h * 128:(h + 1) * 128], rhs=vT[:],
                        start=True, stop=True)
                    nc.scalar.activation(
                        out=h_sb[:, h * 512:(h + 1) * 512], in_=h_ps[:],
                        func=AF.Prelu, bias=b1sb[:, h:h + 1], scale=1.0, alpha=0.01)
                f_ps = psF.tile([64, 512], F32, tag="fp")
                nc.tensor.matmul(out=f_ps[:], lhsT=w2sb[:, 0:64], rhs=h_sb[:, 0:512],
                                 start=True, stop=False)
                nc.tensor.matmul(out=f_ps[:], lhsT=w2sb[:, 64:128], rhs=h_sb[:, 512:1024],
                                 start=False, stop=True)
                f_sb = ep.tile([64, 512], F32, tag="fsb")
                nc.scalar.activation(out=f_sb[:], in_=f_ps[:], func=AF.Identity,
                                     bias=b2sb[:, 0:1])
                Fo_ps = psT.tile([128, 256], F32, tag="tp")
                for j in range(4):
                    nc.tensor.transpose(
                        out=Fo_ps[:, j * 64:(j + 1) * 64],
                        in_=f_sb[:, j * 128:(j + 1) * 128],
                        identity=ident[:64, :64])
                nc.vector.tensor_copy(out=F_all[:, c * 256:(c + 1) * 256], in_=Fo_ps[:])

            def ph2_tile(bt):
                iu = pp.tile([128, G], I32, tag="iu", bufs=6)
                nc.sync.dma_start(out=iu[:], in_=iu_d[bt])
                ip = pp.tile([128, 1], I32, tag="ip")
                nc.sync.dma_start(out=ip[:], in_=ip_d[bt])
                mk = pp.tile([128, G], I32, tag="mk")
                nc.sync.dma_start(out=mk[:], in_=mk_d[bt])

                Ug = pp.tile([128, G * 64], BF16, tag="Ug", bufs=5)
                for g in range(G):
                    nc.gpsimd.indirect_dma_start(
                        out=Ug[:, g * 64:(g + 1) * 64], out_offset=None, in_=id_d[:],
                        in_offset=IndirectOffsetOnAxis(ap=iu[:, g:g + 1], axis=0))
                # positive item embedding = first 64 floats of its EV row
                Pg = pp.tile([128, 64], BF16, tag="Pg")
                nc.gpsimd.indirect_dma_start(
                    out=Pg[:], out_offset=None, in_=ev_d[:],
                    in_offset=IndirectOffsetOnAxis(ap=ip[:], axis=0))

                F3 = F_all[:, bt * G * 64:(bt + 1) * G * 64].rearrange(
                    "p (g e) -> p g e", e=64)
                E3 = E_all[:, bt * G * 64:(bt + 1) * G * 64].rearrange(
                    "p (g e) -> p g e", e=64)
                U3 = Ug[:].rearrange("p (g e) -> p g e", e=64)
                P3b = Pg[:].rearrange("p (g e) -> p g e", g=1).to_broadcast([128, G, 64])
                m3b = mk[:].rearrange("p (g o) -> p g o", o=1).to_broadcast([128, G, 64])

                prA = prp.tile([128, G * 64], F32, tag="prA")
                prA3 = prA[:].rearrange("p (g e) -> p g e", e=64)
                dPF = pp.tile([128, G], F32, tag="dPF")
                nc.vector.tensor_tensor(out=prA3, in0=F3, in1=P3b, op=ALU.mult)
                nc.vector.tensor_reduce(out=dPF[:], in_=prA3, op=ALU.add, axis=AX.X)

                prB = prp.tile([128, G * 64], F32, tag="prB")
                prB3 = prB[:].rearrange("p (g e) -> p g e", e=64)
                dFF = pp.tile([128, G], F32, tag="dFF")
                nc.vector.tensor_tensor(out=prB3, in0=F3, in1=F3, op=ALU.mult)
                nc.vector.tensor_reduce(out=dFF[:], in_=prB3, op=ALU.add, axis=AX.X)

                prP = pp.tile([128, 64], F32, tag="prP")
                dPP = pp.tile([128, 1], F32, tag="dPP")
                nc.vector.tensor_tensor(out=prP[:], in0=Pg[:], in1=Pg[:], op=ALU.mult)
                nc.vector.tensor_reduce(
                    out=dPP[:], in_=prP[:].rearrange("p (o e) -> p o e", o=1),
                    op=ALU.add, axis=AX.X)

                # all_item_input rows: replace masked rows of E with F (in place)
                nc.vector.copy_predicated(out=E3, mask=m3b, data=F3)
                prC = prp.tile([128, G * 64], F32, tag="prC")
                prC3 = prC[:].rearrange("p (g e) -> p g e", e=64)
                d2 = pp.tile([128, G], F32, tag="d2")
                nc.vector.tensor_tensor(out=prC3, in0=U3, in1=E3, op=ALU.mult)
                nc.vector.tensor_reduce(out=d2[:], in_=prC3, op=ALU.add, axis=AX.X)

                # inverse norms (overwrite dFF / dPP in place)
                nc.vector.tensor_scalar_max(out=dFF[:], in0=dFF[:], scalar1=1e-24)
                nc.scalar.activation(out=dFF[:], in_=dFF[:], func=AF.Ln)
                nc.scalar.activation(out=dFF[:], in_=dFF[:], func=AF.Exp, scale=-0.5)
                nc.vector.tensor_scalar_max(out=dPP[:], in0=dPP[:], scalar1=1e-24)
                nc.scalar.activation(out=dPP[:], in_=dPP[:], func=AF.Ln)
                nc.scalar.activation(out=dPP[:], in_=dPP[:], func=AF.Exp, scale=-0.5)

                d1 = pp.tile([128, G], F32, tag="d1")
                nc.vector.tensor_tensor(out=d1[:], in0=dPF[:], in1=dFF[:], op=ALU.mult)
                nc.vector.tensor_scalar_mul(out=d1[:], in0=d1[:], scalar1=dPP[:, 0:1])

                s1 = pp.tile([128, G], F32, tag="s1")
                s2 = pp.tile([128, G], F32, tag="s2")
                tot = pp.tile([128, 2], F32, tag="tot")
                nc.scalar.activation(out=s1[:], in_=d1[:], func=AF.Exp,
                                     scale=1.0 / TEMP, accum_out=tot[:, 0:1])
                nc.scalar.activation(out=s2[:], in_=d2[:], func=AF.Exp,
                                     scale=1.0 / TEMP, accum_out=tot[:, 1:2])
                nc.vector.tensor_scalar_add(out=tot[:], in0=tot[:], scalar1=1e-8)
                rc = pp.tile([128, 2], F32, tag="rc")
                nc.vector.reciprocal(out=rc[:], in_=tot[:])
                ratio = pp.tile([128, 2], F32, tag="ratio")
                nc.vector.tensor_tensor(out=ratio[:, 0:1], in0=s1[:, 0:1],
                                        in1=rc[:, 0:1], op=ALU.mult)
                nc.vector.tensor_tensor(out=ratio[:, 1:2], in0=s2[:, 0:1],
                                        in1=rc[:, 1:2], op=ALU.mult)
                nc.vector.tensor_scalar_add(out=ratio[:], in0=ratio[:], scalar1=1e-8)
                nc.scalar.activation(out=ratio[:], in_=ratio[:], func=AF.Ln)
                nc.vector.tensor_tensor(out=acc[:], in0=acc[:], in1=ratio[:], op=ALU.add)
                if DEBUG:
                    nc.sync.dma_start(out=dbg1_d[bt], in_=d1[:])
                    nc.sync.dma_start(out=dbg2_d[bt], in_=d2[:])

            next_c = 0
            for bt in range(NT):
                need = -(-(bt + 3) * G // 4)  # ceil; lead 2 batch-tiles
                while next_c < min(need, NCH):
                    enc_chunk(next_c)
                    next_c += 1
                ph2_tile(bt)
            while next_c < NCH:
                enc_chunk(next_c)
                next_c += 1

            nc.sync.dma_start(out=out_d[:], in_=acc[:])
            if DEBUG:
                nc.gpsimd.dma_start(out=dbgF_d[:], in_=F_all[:])

    nc.compile()
    return nc


def _get_nc():
    if "nc" not in _CACHE:
        _CACHE["nc"] = _build()
    return _CACHE["nc"]


def _host_prep(user_tensor, item_tensor, rand_index):
    it = np.ascontiguousarray(item_tensor.astype(np.int32))
    ut = np.ascontiguousarray(user_tensor.astype(np.int32))
    item_idx = np.clip(it - NUM_USER, 0, NUM_ITEM - 1).astype(np.int32)
    mask = np.zeros(B * G, np.int32)
    mask[np.asarray(rand_index, dtype=np.int64)] = 1
    mask = mask.reshape(B, G)
    per_core = []
    for k in range(NCORE):
        sl = slice(k * BC, (k + 1) * BC)
        idxu = np.ascontiguousarray(ut[sl].reshape(NT, 128, G))
        idxp = np.ascontiguousarray(item_idx[sl, 0].reshape(NT, 128, 1))
        mk = np.ascontiguousarray(mask[sl].reshape(NT, 128, G))
        itk = item_idx[sl].reshape(NT, 128, G)
        blocks = itk.transpose(0, 2, 1).reshape(CB, 128)  # [q, p], q = bt*G + g
        idxv = np.ascontiguousarray(
            blocks.reshape(NCH, 4, 128).transpose(0, 2, 1))  # [c, p, j]
        per_core.append((idxv, idxu, idxp, mk))
    return per_core


def kernel(v_feat, id_embedding, W1, b1, W2, b2, user_tensor, item_tensor,
           rand_index):
    nc = _get_nc()
    v_feat = np.asarray(v_feat, dtype=np.float32)
    id_embedding = np.asarray(id_embedding, dtype=np.float32)
    evtab = np.ascontiguousarray(
        np.concatenate([id_embedding[NUM_USER:], v_feat], axis=1))
    iduser = np.ascontiguousarray(id_embedding[:NUM_USER])
    W1 = np.ascontiguousarray(W1, dtype=np.float32)
    b1 = np.ascontiguousarray(b1, dtype=np.float32)
    W2 = np.ascontiguousarray(W2, dtype=np.float32)
    b2 = np.ascontiguousarray(b2, dtype=np.float32)
    per_core = _host_prep(user_tensor, item_tensor, rand_index)
    in_maps = []
    for k in range(NCORE):
        idxv, idxu, idxp, mk = per_core[k]
        in_maps.append({
            "evtab": evtab, "iduser": iduser,
            "w1": W1, "b1": b1, "w2": W2, "b2": b2,
            "idxv": idxv, "idxu": idxu, "idxp": idxp,
            "mask": mk,
        })
    trace = bool(int(os.environ.get("KERNEL_TRACE", "0")))
    res = bass_utils.run_bass_kernel_spmd(
        nc, in_maps, core_ids=list(range(NCORE)), trace=trace)
    _CACHE["last_results"] = res
    accs = np.stack([r["acc_out"] for r in res.results])  # [8, 128, 2]
    sums = accs.sum(axis=(0, 1), dtype=np.float64)
    l1 = -sums[0] / B
    l2 = -sums[1] / B
    return np.array(LR_LAMBDA * l1 + (1.0 - LR_LAMBDA) * l2, dtype=np.float32)



# revision 1
# speedup vs baseline: 1.4785x; 1.4785x over previous
"""Trainium2 Bass kernel for nn_CLCRec contrastive loss.

Strategy (fully local per core, no collectives):
  - Batch rows are sharded 8 ways (2048 rows/core, 17 group entries each).
  - Host packs a combined per-item table  EV[j] = [id_emb[NUM_USER+j] | v_feat[j]]
    (192 f32 = 768B rows) so one indirect-DMA row fetch supplies both the raw
    item embedding (E) and the encoder input (v) for that batch entry.
  - Each core gathers EV rows for its batch entries, runs the MLP encoder on
    the gathered v-halves (PE matmuls), and keeps encoded features + E rows
    in SBUF (bf16).
  - User / positive-item embedding rows are gathered with indirect DMA (bf16).
  - Row-wise dot products + norms feed the two contrastive losses; per-core
    partial sums of the per-row log terms are returned and reduced on host.

Hardware notes baked in here:
  - Indirect DMA honours exactly ONE index per destination partition; multi-
    index offset tiles silently fetch consecutive rows.  All gathers use
    offset tiles of shape [128, 1].
  - All ACT functions used (Ln/Exp/Lrelu/Copy/Identity) live in the
    natural_log_exp_and_others table set; get_activation_tables is patched so
    the table-load pass cannot pick the per-anchor sets (which caused 261
    ACT_TABLE_LOADs, ~335us, in the unpatched build).
"""

import os
import sys

import numpy as np

for _p in ("/opt/trn_rl_repo", os.path.expanduser("~/.axon_site/_ro/trn_rl_repo")):
    if os.path.isdir(_p) and _p not in sys.path:
        sys.path.insert(0, _p)

import concourse.bacc as bacc
import concourse.mybir as mybir
import concourse.tile as tile
from concourse import bass_utils
from concourse.bass import IndirectOffsetOnAxis
from concourse.masks import make_identity

F32 = mybir.dt.float32
BF16 = mybir.dt.bfloat16
I32 = mybir.dt.int32
AF = mybir.ActivationFunctionType
ALU = mybir.AluOpType
AX = mybir.AxisListType

NUM_USER = 200000
NUM_ITEM = 200000
DIM_E = 64
DIM_FEAT = 128
EVW = DIM_E + DIM_FEAT   # 192 floats per packed EV row
B = 16384
G = 17  # 1 + num_neg
TEMP = 0.2
LR_LAMBDA = 0.5

NCORE = 8
BC = B // NCORE          # 2048 batch rows per core
NT = BC // 128           # 16 batch tiles (128 batch rows each)
CB = NT * G              # 272 column blocks of 128 rows per core
NCH = CB // 4            # 68 encoder chunks (512 gathered rows each)

_CACHE: dict = {}

_NEEDED_AF = None  # set lazily (mybir enums)


def _patch_act_tables():
    """Force every activation we emit to resolve to the combined
    natural_log_exp_and_others set so no per-anchor table swapping occurs."""
    global _NEEDED_AF
    if _CACHE.get("act_patched"):
        return
    _NEEDED_AF = {AF.Ln, AF.Exp, AF.Prelu, AF.Copy, AF.Identity}
    import concourse.hw_specs as hw_specs
    orig = hw_specs.get_activation_tables

    def patched(module_arch):
        tabs = orig(module_arch)
        out = {}
        for name, fns in tabs.items():
            if name == "natural_log_exp_and_others":
                out[name] = fns
            else:
                out[name] = fns - _NEEDED_AF
        return out

    bacc.get_activation_tables = patched
    _CACHE["act_patched"] = True


def _build():
    _patch_act_tables()
    nc = bacc.Bacc("TRN2", target_bir_lowering=False, debug=False, num_devices=NCORE)

    ev_d = nc.dram_tensor("evtab", [NUM_ITEM, EVW], F32, kind="ExternalInput")
    id_d = nc.dram_tensor("iduser", [NUM_USER, DIM_E], F32, kind="ExternalInput")
    w1_d = nc.dram_tensor("w1", [DIM_FEAT, 256], F32, kind="ExternalInput")
    b1_d = nc.dram_tensor("b1", [256], F32, kind="ExternalInput")
    w2_d = nc.dram_tensor("w2", [256, DIM_E], F32, kind="ExternalInput")
    b2_d = nc.dram_tensor("b2", [DIM_E], F32, kind="ExternalInput")
    iv_d = nc.dram_tensor("idxv", [NCH, 128, 4], I32, kind="ExternalInput")
    iu_d = nc.dram_tensor("idxu", [NT, 128, G], I32, kind="ExternalInput")
    ip_d = nc.dram_tensor("idxp", [NT, 128, 1], I32, kind="ExternalInput")
    mk_d = nc.dram_tensor("mask", [NT, 128, G], I32, kind="ExternalInput")
    out_d = nc.dram_tensor("acc_out", [128, 2], F32, kind="ExternalOutput")
    DEBUG = bool(int(os.environ.get("KERNEL_DEBUG", "0")))
    if DEBUG:
        dbgF_d = nc.dram_tensor("dbg_f", [128, CB * 64], F32, kind="ExternalOutput")
        dbg1_d = nc.dram_tensor("dbg_d1", [NT, 128, G], F32, kind="ExternalOutput")
        dbg2_d = nc.dram_tensor("dbg_d2", [NT, 128, G], F32, kind="ExternalOutput")

    with tile.TileContext(nc) as tc:
        with tc.tile_pool(name="const", bufs=1) as cp, \
             tc.tile_pool(name="fall", bufs=1) as fp, \
             tc.tile_pool(name="enc", bufs=3) as ep, \
             tc.tile_pool(name="ph2", bufs=3) as pp, \
             tc.tile_pool(name="prd", bufs=3) as prp, \
             tc.tile_pool(name="psT", bufs=3, space="PSUM") as psT, \
             tc.tile_pool(name="psH", bufs=3, space="PSUM") as psH, \
             tc.tile_pool(name="psF", bufs=2, space="PSUM") as psF:

            ident = cp.tile([128, 128], F32, tag="ident")
            make_identity(nc, ident[:])

            w1sb = cp.tile([128, 256], F32, tag="w1sb")
            nc.sync.dma_start(out=w1sb[:], in_=w1_d[:])
            w2sb = cp.tile([128, 128], F32, tag="w2sb")
            nc.sync.dma_start(out=w2sb[:, 0:64], in_=w2_d[0:128, :])
            nc.sync.dma_start(out=w2sb[:, 64:128], in_=w2_d[128:256, :])
            b1sb = cp.tile([128, 2], F32, tag="b1sb")
            nc.sync.dma_start(out=b1sb[:], in_=b1_d[:].rearrange("(h p) -> p h", p=128))
            b2sb = cp.tile([64, 1], F32, tag="b2sb")
            nc.sync.dma_start(out=b2sb[:], in_=b2_d[:].rearrange("(e o) -> e o", o=1))

            # Per-(batch row, group) encoded features and raw item embeddings,
            # kept in SBUF for the whole kernel (bf16 to fit).
            F_all = fp.tile([128, CB * 64], BF16, tag="fall")
            E_all = fp.tile([128, CB * 64], BF16, tag="eall")
            acc = cp.tile([128, 2], F32, tag="acc")
            nc.vector.memset(acc[:], 0.0)

            def enc_chunk(c):
                iv = ep.tile([128, 4], I32, tag="iv", bufs=6)
                nc.sync.dma_start(out=iv[:], in_=iv_d[c])
                ev = ep.tile([128, 4 * EVW], BF16, tag="ev", bufs=5)
                for j in range(4):
                    nc.gpsimd.indirect_dma_start(
                        out=ev[:, j * EVW:(j + 1) * EVW], out_offset=None,
                        in_=ev_d[:],
                        in_offset=IndirectOffsetOnAxis(ap=iv[:, j:j + 1], axis=0))
                ev3 = ev[:].rearrange("p (j e) -> p j e", e=EVW)
                v3 = ev3[:, :, DIM_E:EVW]          # [128, 4, 128] strided
                # stash raw item embedding halves (bf16) for the l2 loss
                nc.scalar.activation(
                    out=E_all[:, c * 256:(c + 1) * 256].rearrange(
                        "p (j e) -> p j e", e=64),
                    in_=ev3[:, :, 0:DIM_E], func=AF.Copy)
                sq = ep.tile([128, 512], F32, tag="sq")
                ss = ep.tile([128, 4], F32, tag="ss")
                nc.vector.tensor_tensor(
                    out=sq[:].rearrange("p (j e) -> p j e", e=DIM_FEAT),
                    in0=v3, in1=v3, op=ALU.mult)
                nc.vector.tensor_reduce(
                    out=ss[:], in_=sq[:].rearrange("p (j e) -> p j e", e=DIM_FEAT),
                    op=ALU.add, axis=AX.X)
                # 1/max(sqrt(ss), 1e-12) == rsqrt(max(ss, 1e-24)) = exp(-0.5*ln(.))
                nc.vector.tensor_scalar_max(out=ss[:], in0=ss[:], scalar1=1e-24)
                nc.scalar.activation(out=ss[:], in_=ss[:], func=AF.Ln)
                nc.scalar.activation(out=ss[:], in_=ss[:], func=AF.Exp, scale=-0.5)
                vn = ep.tile([128, 512], F32, tag="vn")
                nc.vector.tensor_tensor(
                    out=vn[:].rearrange("p (j e) -> p j e", e=DIM_FEAT),
                    in0=v3,
                    in1=ss[:].rearrange("p (j o) -> p j o", o=1).to_broadcast([128, 4, DIM_FEAT]),
                    op=ALU.mult)
                vT_ps = psT.tile([128, 512], F32, tag="tp")
                for j in range(4):
                    nc.tensor.transpose(
                        out=vT_ps[:, j * 128:(j + 1) * 128],
                        in_=vn[:, j * 128:(j + 1) * 128],
                        identity=ident[:])
                vT = ep.tile([128, 512], F32, tag="vT")
                nc.scalar.copy(out=vT[:], in_=vT_ps[:])
                h_sb = ep.tile([128, 1024], F32, tag="hsb")
                for h in range(2):
                    h_ps = psH.tile([128, 512], F32, tag="hp")
                    nc.tensor.matmul(
                        out=h_ps[:], lhsT=w1sb[:, h * 128:(h + 1) * 128], rhs=vT[:],
                        start=True, stop=True)
                    nc.scalar.activation(
                        out=h_sb[:, h * 512:(h + 1) * 512], in_=h_ps[:],
                        func=AF.Prelu, bias=b1sb[:, h:h + 1], scale=1.0, alpha=0.01)
                f_ps = psF.tile([64, 512], F32, tag="fp")
                nc.tensor.matmul(out=f_ps[:], lhsT=w2sb[:, 0:64], rhs=h_sb[:, 0:512],
                                 start=True, stop=False)
                nc.tensor.matmul(out=f_ps[:], lhsT=w2sb[:, 64:128], rhs=h_sb[:, 512:1024],
                                 start=False, stop=True)
                f_sb = ep.tile([64, 512], F32, tag="fsb")
                nc.scalar.activation(out=f_sb[:], in_=f_ps[:], func=AF.Identity,
                                     bias=b2sb[:, 0:1])
                Fo_ps = psT.tile([128, 256], F32, tag="tp")
                for j in range(4):
                    nc.tensor.transpose(
                        out=Fo_ps[:, j * 64:(j + 1) * 64],
                        in_=f_sb[:, j * 128:(j + 1) * 128],
                        identity=ident[:64, :64])
                nc.vector.tensor_copy(out=F_all[:, c * 256:(c + 1) * 256], in_=Fo_ps[:])

            def ph2_tile(bt):
                iu = pp.tile([128, G], I32, tag="iu", bufs=6)
                nc.sync.dma_start(out=iu[:], in_=iu_d[bt])
                ip = pp.tile([128, 1], I32, tag="ip")
                nc.sync.dma_start(out=ip[:], in_=ip_d[bt])
                mk = pp.tile([128, G], I32, tag="mk")
                nc.sync.dma_start(out=mk[:], in_=mk_d[bt])

                Ug = pp.tile([128, G * 64], BF16, tag="Ug", bufs=5)
                for g in range(G):
                    nc.gpsimd.indirect_dma_start(
                        out=Ug[:, g * 64:(g + 1) * 64], out_offset=None, in_=id_d[:],
                        in_offset=IndirectOffsetOnAxis(ap=iu[:, g:g + 1], axis=0))
                # positive item embedding = first 64 floats of its EV row
                Pg = pp.tile([128, 64], BF16, tag="Pg")
                nc.gpsimd.indirect_dma_start(
                    out=Pg[:], out_offset=None, in_=ev_d[:],
                    in_offset=IndirectOffsetOnAxis(ap=ip[:], axis=0))

                F3 = F_all[:, bt * G * 64:(bt + 1) * G * 64].rearrange(
                    "p (g e) -> p g e", e=64)
                E3 = E_all[:, bt * G * 64:(bt + 1) * G * 64].rearrange(
                    "p (g e) -> p g e", e=64)
                U3 = Ug[:].rearrange("p (g e) -> p g e", e=64)
                P3b = Pg[:].rearrange("p (g e) -> p g e", g=1).to_broadcast([128, G, 64])
                m3b = mk[:].rearrange("p (g o) -> p g o", o=1).to_broadcast([128, G, 64])

                prA = prp.tile([128, G * 64], F32, tag="prA")
                prA3 = prA[:].rearrange("p (g e) -> p g e", e=64)
                dPF = pp.tile([128, G], F32, tag="dPF")
                nc.vector.tensor_tensor(out=prA3, in0=F3, in1=P3b, op=ALU.mult)
                nc.vector.tensor_reduce(out=dPF[:], in_=prA3, op=ALU.add, axis=AX.X)

                prB = prp.tile([128, G * 64], F32, tag="prB")
                prB3 = prB[:].rearrange("p (g e) -> p g e", e=64)
                dFF = pp.tile([128, G], F32, tag="dFF")
                nc.vector.tensor_tensor(out=prB3, in0=F3, in1=F3, op=ALU.mult)
                nc.vector.tensor_reduce(out=dFF[:], in_=prB3, op=ALU.add, axis=AX.X)

                prP = pp.tile([128, 64], F32, tag="prP")
                dPP = pp.tile([128, 1], F32, tag="dPP")
                nc.vector.tensor_tensor(out=prP[:], in0=Pg[:], in1=Pg[:], op=ALU.mult)
                nc.vector.tensor_reduce(
                    out=dPP[:], in_=prP[:].rearrange("p (o e) -> p o e", o=1),
                    op=ALU.add, axis=AX.X)

                # all_item_input rows: replace masked rows of E with F (in place)
                nc.vector.copy_predicated(out=E3, mask=m3b, data=F3)
                prC = prp.tile([128, G * 64], F32, tag="prC")
                prC3 = prC[:].rearrange("p (g e) -> p g e", e=64)
                d2 = pp.tile([128, G], F32, tag="d2")
                nc.vector.tensor_tensor(out=prC3, in0=U3, in1=E3, op=ALU.mult)
                nc.vector.tensor_reduce(out=d2[:], in_=prC3, op=ALU.add, axis=AX.X)

                # inverse norms (overwrite dFF / dPP in place)
                nc.vector.tensor_scalar_max(out=dFF[:], in0=dFF[:], scalar1=1e-24)
                nc.scalar.activation(out=dFF[:], in_=dFF[:], func=AF.Ln)
                nc.scalar.activation(out=dFF[:], in_=dFF[:], func=AF.Exp, scale=-0.5)
                nc.vector.tensor_scalar_max(out=dPP[:], in0=dPP[:], scalar1=1e-24)
                nc.scalar.activation(out=dPP[:], in_=dPP[:], func=AF.Ln)
                nc.scalar.activation(out=dPP[:], in_=dPP[:], func=AF.Exp, scale=-0.5)

                d1 = pp.tile([128, G], F32, tag="d1")
                nc.vector.tensor_tensor(out=d1[:], in0=dPF[:], in1=dFF[:], op=ALU.mult)
                nc.vector.tensor_scalar_mul(out=d1[:], in0=d1[:], scalar1=dPP[:, 0:1])

                s1 = pp.tile([128, G], F32, tag="s1")
                s2 = pp.tile([128, G], F32, tag="s2")
                tot = pp.tile([128, 2], F32, tag="tot")
                nc.scalar.activation(out=s1[:], in_=d1[:], func=AF.Exp,
                                     scale=1.0 / TEMP, accum_out=tot[:, 0:1])
                nc.scalar.activation(out=s2[:], in_=d2[:], func=AF.Exp,
                                     scale=1.0 / TEMP, accum_out=tot[:, 1:2])
                nc.vector.tensor_scalar_add(out=tot[:], in0=tot[:], scalar1=1e-8)
                rc = pp.tile([128, 2], F32, tag="rc")
                nc.vector.reciprocal(out=rc[:], in_=tot[:])
                ratio = pp.tile([128, 2], F32, tag="ratio")
                nc.vector.tensor_tensor(out=ratio[:, 0:1], in0=s1[:, 0:1],
                                        in1=rc[:, 0:1], op=ALU.mult)
                nc.vector.tensor_tensor(out=ratio[:, 1:2], in0=s2[:, 0:1],
                                        in1=rc[:, 1:2], op=ALU.mult)
                nc.vector.tensor_scalar_add(out=ratio[:], in0=ratio[:], scalar1=1e-8)
                nc.scalar.activation(out=ratio[:], in_=ratio[:], func=AF.Ln)
                nc.vector.tensor_tensor(out=acc[:], in0=acc[:], in1=ratio[:], op=ALU.add)
                if DEBUG:
                    nc.sync.dma_start(out=dbg1_d[bt], in_=d1[:])
                    nc.sync.dma_start(out=dbg2_d[bt], in_=d2[:])

            next_c = 0
            for bt in range(NT):
                need = -(-(bt + 3) * G // 4)  # ceil; lead 2 batch-tiles
                while next_c < min(need, NCH):
                    enc_chunk(next_c)
                    next_c += 1
                ph2_tile(bt)
            while next_c < NCH:
                enc_chunk(next_c)
                next_c += 1

            nc.sync.dma_start(out=out_d[:], in_=acc[:])
            if DEBUG:
                nc.gpsimd.dma_start(out=dbgF_d[:], in_=F_all[:])

    nc.compile()
    return nc


def _get_nc():
    if "nc" not in _CACHE:
        _CACHE["nc"] = _build()
    return _CACHE["nc"]


def _host_prep(user_tensor, item_tensor, rand_index):
    it = np.ascontiguousarray(item_tensor.astype(np.int32))
    ut = np.ascontiguousarray(user_tensor.astype(np.int32))
    item_idx = np.clip(it - NUM_USER, 0, NUM_ITEM - 1).astype(np.int32)
    mask = np.zeros(B * G, np.int32)
    mask[np.asarray(rand_index, dtype=np.int64)] = 1
    mask = mask.reshape(B, G)
    per_core = []
    for k in range(NCORE):
        sl = slice(k * BC, (k + 1) * BC)
        idxu = np.ascontiguousarray(ut[sl].reshape(NT, 128, G))
        idxp = np.ascontiguousarray(item_idx[sl, 0].reshape(NT, 128, 1))
        mk = np.ascontiguousarray(mask[sl].reshape(NT, 128, G))
        itk = item_idx[sl].reshape(NT, 128, G)
        blocks = itk.transpose(0, 2, 1).reshape(CB, 128)  # [q, p], q = bt*G + g
        idxv = np.ascontiguousarray(
            blocks.reshape(NCH, 4, 128).transpose(0, 2, 1))  # [c, p, j]
        per_core.append((idxv, idxu, idxp, mk))
    return per_core


def kernel(v_feat, id_embedding, W1, b1, W2, b2, user_tensor, item_tensor,
           rand_index):
    nc = _get_nc()
    v_feat = np.asarray(v_feat, dtype=np.float32)
    id_embedding = np.asarray(id_embedding, dtype=np.float32)
    evtab = np.ascontiguousarray(
        np.concatenate([id_embedding[NUM_USER:], v_feat], axis=1))
    iduser = np.ascontiguousarray(id_embedding[:NUM_USER])
    W1 = np.ascontiguousarray(W1, dtype=np.float32)
    b1 = np.ascontiguousarray(b1, dtype=np.float32)
    W2 = np.ascontiguousarray(W2, dtype=np.float32)
    b2 = np.ascontiguousarray(b2, dtype=np.float32)
    per_core = _host_prep(user_tensor, item_tensor, rand_index)
    in_maps = []
    for k in range(NCORE):
        idxv, idxu, idxp, mk = per_core[k]
        in_maps.append({
            "evtab": evtab, "iduser": iduser,
            "w1": W1, "b1": b1, "w2": W2, "b2": b2,
            "idxv": idxv, "idxu": idxu, "idxp": idxp,
            "mask": mk,
        })
    trace = bool(int(os.environ.get("KERNEL_TRACE", "0")))
    res = bass_utils.run_bass_kernel_spmd(
        nc, in_maps, core_ids=list(range(NCORE)), trace=trace)
    _CACHE["last_results"] = res
    accs = np.stack([r["acc_out"] for r in res.results])  # [8, 128, 2]
    sums = accs.sum(axis=(0, 1), dtype=np.float64)
    l1 = -sums[0] / B
    l2 = -sums[1] / B
    return np.array(LR_LAMBDA * l1 + (1.0 - LR_LAMBDA) * l2, dtype=np.float32)

